# revision 8
# baseline (speedup 1.0000x reference)
"""EuclideanTransformer message-passing kernel for 8 trn2 cores.

Sharding: edges are partitioned by receiver block (2048 nodes/core, 16
blocks of 128 nodes per core), sorted by receiver, and padded so every
core has an identical static tile structure (T_BLK 128-edge tiles per
block). Node k/v projections are computed on-device (replicated) into a
DRAM gather table; per-edge sender rows come via indirect DMA. Receiver
q/qev values are computed per block and expanded to edges with a
one-hot matmul, so the segment-sum accumulates in PSUM per block and no
collective is needed: each core owns its receivers' output rows and the
host concatenates.
"""

import numpy as np
import ml_dtypes

import concourse.bass as bass
import concourse.mybir as mybir
from concourse.tile import TileContext
from concourse.tile_rust import add_dep_helper
from concourse.bass_utils import run_bass_kernel_spmd

BF = mybir.dt.bfloat16
F32 = mybir.dt.float32
I32 = mybir.dt.int32

N = 16384
E = 262144
F = 128
H = 8
DH = 16
L = 4
DE = 32
EV = 16
NRB = 32
SEG = [0, 1, 4, 9, 16]
NC = 8
NPC = N // NC          # nodes per core
NBLK = NPC // 128      # blocks of 128 nodes per core
ATT_I = float(np.sqrt(DH))
ATT_E = float(np.sqrt(DE))

TS_W = 400             # sender table row: k(128)|kev(128)|v(128)|ev(16)
PK = 20                # edgepack cols: recv_local, cutoff, sh(16), pad(2)


def _fix_multiwait(nc):
    """This walrus build accepts a single sync-wait per instruction; Tile
    emits several.  Hoist extras onto same-engine NoOps."""
    ctr = 0
    for f in nc.m.functions:
        for bb in f.blocks:
            new = []
            for inst in bb.instructions:
                si = getattr(inst, "sync_info", None)
                ow = list(si.on_wait) if si and si.on_wait else []
                if len(ow) > 1:
                    for w in ow[:-1]:
                        ctr += 1
                        nop = mybir.InstNoOp(
                            name=f"waitnop-{ctr}", ins=[], outs=[], engine=inst.engine
                        )
                        nop.sync_info = mybir.SyncInfo(on_wait=[w], on_update=[])
                        new.append(nop)
                    si.on_wait = [ow[-1]]
                new.append(inst)
            bb.instructions[:] = new


def build_program(t_blk):
    ep = NBLK * t_blk * 128          # padded edges per core
    AF = mybir.ActivationFunctionType

    nc = bass.Bass()
    dt = nc.dram_tensor
    senders_d = dt("senders", [128, ep // 128], I32, kind="ExternalInput")
    epack_d = dt("epack", [128, (ep // 128) * PK], F32, kind="ExternalInput")
    rbft_d = dt("rbft", [NRB, ep], BF, kind="ExternalInput")
    xt_d = dt("xt", [128, N], BF, kind="ExternalInput")
    xtc_d = dt("xtc", [128, NPC], BF, kind="ExternalInput")
    evbf_d = dt("evbf", [128, (N // 128) * EV], BF, kind="ExternalInput")
    invblk_d = dt("invblk", [NPC, F], F32, kind="ExternalInput")
    evblk_d = dt("evblk", [NPC, EV], F32, kind="ExternalInput")
    wq_d = dt("wq", [128, 128], BF, kind="ExternalInput")
    wk_d = dt("wk", [128, 128], BF, kind="ExternalInput")
    wv_d = dt("wv", [128, 128], BF, kind="ExternalInput")
    wqe_d = dt("wqe", [128, 128], BF, kind="ExternalInput")
    wke_d = dt("wke", [128, 128], BF, kind="ExternalInput")
    w1r_d = dt("w1r", [NRB, 256], BF, kind="ExternalInput")
    w1e_d = dt("w1e", [L, 64], BF, kind="ExternalInput")
    w2r_d = dt("w2r", [128, 256], BF, kind="ExternalInput")
    w2e_d = dt("w2e", [33, 256], BF, kind="ExternalInput")
    b1r_d = dt("b1r", [128, 2], F32, kind="ExternalInput")
    b1e_d = dt("b1e", [32, 2], F32, kind="ExternalInput")
    l0m_d = dt("l0m", [EV, L], BF, kind="ExternalInput")
    repi_d = dt("repi", [H, 128], BF, kind="ExternalInput")
    repe_d = dt("repe", [L, EV], BF, kind="ExternalInput")
    intwh_d = dt("intwh", [128, F + L], BF, kind="ExternalInput")
    intwl_d = dt("intwl", [33, F + L], BF, kind="ExternalInput")
    iden_d = dt("iden", [128, 128], BF, kind="ExternalInput")
    iotar_d = dt("iotar", [128, 128], BF, kind="ExternalInput")
    iotac_d = dt("iotac", [128, 1], BF, kind="ExternalInput")
    ts_d = dt("tstab", [N, TS_W], BF)
    oinv_d = dt("oinv", [NPC, F], F32, kind="ExternalOutput")
    oev_d = dt("oev", [NPC, EV], F32, kind="ExternalOutput")

    with TileContext(nc) as tc:
        with (
            tc.tile_pool(name="const", bufs=1) as cp,
            tc.tile_pool(name="sb", bufs=2) as sb,
            tc.tile_pool(name="io", bufs=2) as iop,
            tc.tile_pool(name="ps", bufs=1, space="PSUM") as ps,
        ):
            def cload(dram, shape, dtype):
                t = cp.tile(shape, dtype, tag=dram.name)
                nc.sync.dma_start(t[:], dram[:])
                return t

            wq = cload(wq_d, [128, 128], BF)
            wk = cload(wk_d, [128, 128], BF)
            wv = cload(wv_d, [128, 128], BF)
            wqe = cload(wqe_d, [128, 128], BF)
            wke = cload(wke_d, [128, 128], BF)
            w1r = cload(w1r_d, [NRB, 256], BF)
            w1e = cload(w1e_d, [L, 64], BF)
            w2r = cload(w2r_d, [128, 256], BF)
            w2e = cload(w2e_d, [33, 256], BF)
            b1r = cload(b1r_d, [128, 2], F32)
            b1e = cload(b1e_d, [32, 2], F32)
            l0m = cload(l0m_d, [EV, L], BF)
            repi = cload(repi_d, [H, 128], BF)
            repe = cload(repe_d, [L, EV], BF)
            intwh = cload(intwh_d, [128, F + L], BF)
            intwl = cload(intwl_d, [33, F + L], BF)
            iden = cload(iden_d, [128, 128], BF)
            iotar = cload(iotar_d, [128, 128], BF)
            iotac = cload(iotac_d, [128, 1], BF)
            xtc = cload(xtc_d, [128, NPC], BF)

            xt = cp.tile([128, N], BF, tag="xt_t")
            for j in range(8):
                nc.sync.dma_start(
                    xt[:, j * (N // 8):(j + 1) * (N // 8)],
                    xt_d[:, j * (N // 8):(j + 1) * (N // 8)],
                )

            # ---------- phase A: sender projection table ----------
            evb = cp.tile([128, (N // 128) * EV], BF, tag="evb_t")
            nc.sync.dma_start(evb[:], evbf_d[:])
            ts_writes = []
            ts_writes.append(nc.sync.dma_start(
                ts_d[:, 384:400].rearrange("(s p) c -> p s c", p=128),
                evb[:].rearrange("p (s c) -> p s c", c=EV),
            ))
            for g in range(N // 1024):
                tsa = sb.tile([128, 8, 384], BF, tag="tsa")
                for s in range(8):
                    nt = g * 8 + s
                    xsl = xt[:, nt * 128:(nt + 1) * 128]
                    pts = ps.tile([128, 384], F32, tag="pfw")
                    nc.tensor.matmul(out=pts[:, 0:128], lhsT=xsl, rhs=wk, start=True, stop=True)
                    nc.tensor.matmul(out=pts[:, 128:256], lhsT=xsl, rhs=wke, start=True, stop=True)
                    nc.tensor.matmul(out=pts[:, 256:384], lhsT=xsl, rhs=wv, start=True, stop=True)
                    nc.vector.tensor_copy(out=tsa[:, s, :], in_=pts[:])
                ts_writes.append(nc.sync.dma_start(
                    ts_d[g * 1024:(g + 1) * 1024, 0:384].rearrange("(s p) c -> p s c", p=128),
                    tsa[:],
                ))

            # ---------- phase B: edges ----------
            for b in range(NBLK):
                # receiver q|qev for this block, computed in place
                ptrb = ps.tile([128, 256], F32, tag="pexp")
                xcs = xtc[:, b * 128:(b + 1) * 128]
                nc.tensor.matmul(out=ptrb[:, 0:128], lhsT=xcs, rhs=wq, start=True, stop=True)
                nc.tensor.matmul(out=ptrb[:, 128:256], lhsT=xcs, rhs=wqe, start=True, stop=True)
                trch = sb.tile([128, 272], BF, tag="trch")
                nc.vector.tensor_copy(out=trch[:, 0:256], in_=ptrb[:])
                evbk = sb.tile([128, EV], F32, tag="evbk")
                nc.sync.dma_start(evbk[:], evblk_d[b * 128:(b + 1) * 128, :])
                nc.vector.tensor_copy(out=trch[:, 256:272], in_=evbk[:])

                acc = ps.tile([128, 144], F32, tag="acc")
                for tt in range(t_blk):
                    t = b * t_blk + tt
                    ch, s = t // 8, t % 8
                    if s == 0:
                        epk = iop.tile([128, 8, PK], F32, tag="epk")
                        nc.sync.dma_start(
                            epk[:],
                            epack_d[:, ch * 8 * PK:(ch + 1) * 8 * PK]
                            .rearrange("p (s c) -> p s c", c=PK),
                        )
                        snd = iop.tile([128, 8], I32, tag="snd")
                        nc.sync.dma_start(snd[:], senders_d[:, ch * 8:(ch + 1) * 8])
                        rbft = iop.tile([NRB, 1024], BF, tag="rbft")
                        nc.sync.dma_start(rbft[:], rbft_d[:, ch * 1024:(ch + 1) * 1024])

                    cut = epk[:, s, 1:2]
                    shn = epk[:, s, 2:18]
                    rbf_t = rbft[:, s * 128:(s + 1) * 128]

                    tsg = sb.tile([128, TS_W], BF, tag="tsg")
                    g_inst = nc.gpsimd.indirect_dma_start(
                        out=tsg[:], out_offset=None, in_=ts_d[:],
                        in_offset=bass.IndirectOffsetOnAxis(ap=snd[:, s:s + 1], axis=0),
                    )
                    for _w in ts_writes:
                        add_dep_helper(g_inst.ins, _w.ins, reason="gather after table write")

                    recvb = sb.tile([128, 1], BF, tag="recvb")
                    nc.vector.tensor_copy(out=recvb[:], in_=epk[:, s, 0:1])
                    ptb = ps.tile([128, 512], BF, tag="ptb")
                    nc.tensor.transpose(
                        out=ptb[:, 0:128], in_=recvb[:].to_broadcast([128, 128]),
                        identity=iden[:],
                    )
                    oh = sb.tile([128, 128], BF, tag="oh")
                    nc.vector.tensor_tensor(
                        out=oh[:], in0=recvb[:].to_broadcast([128, 128]), in1=iotar[:],
                        op=mybir.AluOpType.is_equal,
                    )
                    oht = sb.tile([128, 128], BF, tag="oht")
                    nc.vector.tensor_tensor(
                        out=oht[:], in0=iotac[:].to_broadcast([128, 128]),
                        in1=ptb[:, 0:128], op=mybir.AluOpType.is_equal,
                    )

                    pexp = ps.tile([128, 272], F32, tag="pexp")
                    nc.tensor.matmul(out=pexp[:], lhsT=oht[:], rhs=trch[:], start=True, stop=True)

                    ph1 = ps.tile([128, 384], F32, tag="ph1")
                    nc.tensor.matmul(out=ph1[:, 0:128], lhsT=w1r[:, 0:128], rhs=rbf_t, start=True, stop=True)
                    nc.tensor.matmul(out=ph1[:, 128:256], lhsT=w1r[:, 128:256], rhs=rbf_t, start=True, stop=True)
                    sfi = sb.tile([128, 128], BF, tag="sfi")
                    nc.scalar.activation(out=sfi[:], in_=ph1[:, 0:128], func=AF.Silu, bias=b1r[:, 0:1])
                    sfe = sb.tile([128, 128], BF, tag="sfe")
                    nc.scalar.activation(out=sfe[:], in_=ph1[:, 128:256], func=AF.Silu, bias=b1r[:, 1:2])

                    evd = sb.tile([128, EV], BF, tag="evd")
                    nc.vector.tensor_tensor(out=evd[:], in0=tsg[:, 384:400],
                                            in1=pexp[:, 256:272], op=mybir.AluOpType.subtract)
                    sq = sb.tile([128, EV], BF, tag="sq")
                    nc.vector.tensor_tensor(out=sq[:], in0=evd[:], in1=evd[:],
                                            op=mybir.AluOpType.mult)
                    nc.tensor.transpose(out=ptb[0:EV, 128:256], in_=sq[:], identity=iden[:])
                    sqt = sb.tile([EV, 128], BF, tag="sqt")
                    nc.vector.tensor_copy(out=sqt[:], in_=ptb[0:EV, 128:256])
                    paux = ps.tile([128, 128], F32, tag="paux")
                    nc.tensor.matmul(out=paux[0:L, :], lhsT=l0m[:], rhs=sqt[:], start=True, stop=True)
                    l0t = sb.tile([L, 128], BF, tag="l0t")
                    nc.vector.tensor_copy(out=l0t[:], in_=paux[0:L, :])
                    nc.tensor.matmul(out=ph1[0:64, 256:384], lhsT=w1e[:], rhs=l0t[:], start=True, stop=True)
                    sev = sb.tile([33, 256], BF, tag="sev")
                    nc.scalar.activation(out=sev[0:32, 0:128], in_=ph1[0:32, 256:384],
                                         func=AF.Silu, bias=b1e[:, 0:1])
                    nc.scalar.activation(out=sev[0:32, 128:256], in_=ph1[32:64, 256:384],
                                         func=AF.Silu, bias=b1e[:, 1:2])
                    nc.vector.memset(sev[32:33, 0:128], 1.0)
                    nc.vector.memset(sev[32:33, 128:256], 1.0)

                    pfw = ps.tile([128, 512], F32, tag="pfw")
                    nc.tensor.matmul(out=pfw[:, 0:128], lhsT=w2r[:, 0:128], rhs=sfi[:], start=True, stop=False)
                    nc.tensor.matmul(out=pfw[:, 0:128], lhsT=w2e[:, 0:128], rhs=sev[:, 0:128], start=False, stop=True)
                    nc.tensor.matmul(out=pfw[:, 128:256], lhsT=w2r[:, 128:256], rhs=sfe[:], start=True, stop=False)
                    nc.tensor.matmul(out=pfw[:, 128:256], lhsT=w2e[:, 128:256], rhs=sev[:, 128:256], start=False, stop=True)
                    fwi = sb.tile([128, 128], BF, tag="fwi")
                    nc.vector.tensor_copy(out=fwi[:], in_=pfw[:, 0:128])
                    fwe = sb.tile([128, 128], BF, tag="fwe")
                    nc.scalar.activation(out=fwe[:], in_=pfw[:, 128:256], func=AF.Copy)
                    pfn = ps.tile([128, 256], BF, tag="pfn")
                    nc.tensor.transpose(out=pfn[:, 0:128], in_=fwi[:], identity=iden[:])
                    nc.tensor.transpose(out=pfn[:, 128:256], in_=fwe[:], identity=iden[:])

                    t1 = sb.tile([128, 128], BF, tag="t1")
                    nc.vector.tensor_tensor(out=t1[:], in0=tsg[:, 0:128], in1=pexp[:, 0:128],
                                            op=mybir.AluOpType.mult)
                    t2 = sb.tile([128, 128], BF, tag="t2")
                    nc.vector.tensor_tensor(out=t2[:], in0=t1[:], in1=pfn[:, 0:128],
                                            op=mybir.AluOpType.mult)
                    t3 = sb.tile([128, 128], BF, tag="t3")
                    nc.vector.tensor_tensor(out=t3[:], in0=tsg[:, 128:256], in1=pexp[:, 128:256],
                                            op=mybir.AluOpType.mult)
                    t4 = sb.tile([128, 128], BF, tag="t4")
                    nc.vector.tensor_tensor(out=t4[:], in0=t3[:], in1=pfn[:, 128:256],
                                            op=mybir.AluOpType.mult)
                    acat = sb.tile([128, 12], F32, tag="acat")
                    nc.vector.reduce_sum(out=acat[:, 0:8],
                                         in_=t2[:].rearrange("p (h d) -> p h d", d=DH),
                                         axis=mybir.AxisListType.X)
                    nc.vector.reduce_sum(out=acat[:, 8:12],
                                         in_=t4[:].rearrange("p (h d) -> p h d", d=DE),
                                         axis=mybir.AxisListType.X)
                    acsi = sb.tile([128, H], BF, tag="acsi")
                    nc.vector.tensor_scalar_mul(acsi[:], acat[:, 0:8], cut)
                    acse = sb.tile([128, L], BF, tag="acse")
                    nc.vector.tensor_scalar_mul(acse[:], acat[:, 8:12], cut)
                    nc.tensor.transpose(out=ptb[0:H, 256:384], in_=acsi[:], identity=iden[:])
                    nc.tensor.transpose(out=ptb[0:L, 384:512], in_=acse[:], identity=iden[:])
                    ati = sb.tile([H, 128], BF, tag="ati")
                    nc.vector.tensor_copy(out=ati[:], in_=ptb[0:H, 256:384])
                    ate = sb.tile([L, 128], BF, tag="ate")
                    nc.vector.tensor_copy(out=ate[:], in_=ptb[0:L, 384:512])
                    prep = ps.tile([128, 144], F32, tag="prep")
                    nc.tensor.matmul(out=prep[:, 0:128], lhsT=ati[:], rhs=repi[:], start=True, stop=True)
                    nc.tensor.matmul(out=prep[:, 128:144], lhsT=ate[:], rhs=repe[:], start=True, stop=True)
                    scl = sb.tile([128, 144], BF, tag="scl")
                    nc.vector.tensor_tensor(out=scl[:, 0:128], in0=prep[:, 0:128],
                                            in1=tsg[:, 256:384], op=mybir.AluOpType.mult)
                    nc.vector.tensor_tensor(out=scl[:, 128:144], in0=prep[:, 128:144],
                                            in1=shn, op=mybir.AluOpType.mult)
                    nc.tensor.matmul(out=acc[:], lhsT=oh[:], rhs=scl[:],
                                     start=(tt == 0), stop=(tt == t_blk - 1))

                # ---- block tail: residuals + interaction ----
                invb = sb.tile([128, F], F32, tag="invb")
                nc.sync.dma_start(invb[:], invblk_d[b * 128:(b + 1) * 128, :])
                inv1 = sb.tile([128, F], F32, tag="inv1")
                nc.vector.tensor_tensor(out=inv1[:], in0=invb[:], in1=acc[:, 0:128],
                                        op=mybir.AluOpType.add)
                ev1 = sb.tile([128, EV], F32, tag="ev1")
                nc.vector.tensor_tensor(out=ev1[:], in0=evbk[:], in1=acc[:, 128:144],
                                        op=mybir.AluOpType.add)
                inv1b = sb.tile([128, F], BF, tag="inv1b")
                nc.vector.tensor_copy(out=inv1b[:], in_=inv1[:])
                pint = ps.tile([128, 512], BF, tag="ptb")
                nc.tensor.transpose(out=pint[:, 0:128], in_=inv1b[:], identity=iden[:])
                inv1t = sb.tile([128, 128], BF, tag="inv1t")
                nc.vector.tensor_copy(out=inv1t[:], in_=pint[:, 0:128])
                sq2 = sb.tile([128, EV], BF, tag="sq2")
                nc.vector.tensor_tensor(out=sq2[:], in0=ev1[:], in1=ev1[:],
                                        op=mybir.AluOpType.mult)
                nc.tensor.transpose(out=pint[0:EV, 128:256], in_=sq2[:], identity=iden[:])
                sq2t = sb.tile([EV, 128], BF, tag="sq2t")
                nc.vector.tensor_copy(out=sq2t[:], in_=pint[0:EV, 128:256])
                paux2 = ps.tile([128, 128], F32, tag="paux")
                nc.tensor.matmul(out=paux2[0:L, :], lhsT=l0m[:], rhs=sq2t[:], start=True, stop=True)
                l0b = sb.tile([33, 128], BF, tag="l0b")
                nc.vector.memset(l0b[0:32, :], 0.0)
                nc.vector.tensor_copy(out=l0b[0:L, :], in_=paux2[0:L, :])
                nc.vector.memset(l0b[32:33, :], 1.0)
                pt = ps.tile([128, F + L], F32, tag="prep")
                nc.tensor.matmul(out=pt[:], lhsT=inv1t[:], rhs=intwh[:], start=True, stop=False)
                nc.tensor.matmul(out=pt[:], lhsT=l0b[0:33, :], rhs=intwl[:], start=False, stop=True)
                oinv = sb.tile([128, F], F32, tag="oinvt")
                nc.vector.tensor_tensor(out=oinv[:], in0=inv1[:], in1=pt[:, 0:F],
                                        op=mybir.AluOpType.add)
                nc.sync.dma_start(oinv_d[b * 128:(b + 1) * 128, :], oinv[:])
                oev = sb.tile([128, EV], F32, tag="oevt")
                for l in range(L):
                    c_l = sb.tile([128, 1], F32, tag=f"cl{l}")
                    nc.scalar.add(out=c_l[:], in_=pt[:, F + l:F + l + 1], add=1.0)
                    nc.vector.tensor_scalar_mul(
                        oev[:, SEG[l]:SEG[l + 1]], ev1[:, SEG[l]:SEG[l + 1]], c_l[:],
                    )
                nc.sync.dma_start(oev_d[b * 128:(b + 1) * 128, :], oev[:])

    _fix_multiwait(nc)
    return nc


def _host_prep(inputs):
    f32 = np.float32
    bf = ml_dtypes.bfloat16
    inv = np.asarray(inputs["inv_features"], f32)
    evf = np.asarray(inputs["ev_features"], f32)
    rbf = np.asarray(inputs["rbf"], f32)
    sh = np.asarray(inputs["sh_vectors"], f32)
    cut = np.asarray(inputs["cutoffs"], f32).reshape(-1)
    snd = np.asarray(inputs["senders"]).astype(np.int64)
    rcv = np.asarray(inputs["receivers"]).astype(np.int64)

    core = rcv // NPC
    blk = (rcv % NPC) // 128
    counts = np.zeros((NC, NBLK), np.int64)
    np.add.at(counts, (core, blk), 1)
    t_blk = int(np.ceil(counts.max() / 128))
    ep = NBLK * t_blk * 128

    order = np.lexsort((rcv,))
    rs, ss = rcv[order], snd[order]
    rbf_s, sh_s, cut_s = rbf[order], sh[order], cut[order]

    senders_a = np.zeros((NC, ep), np.int32)
    epack_a = np.zeros((NC, ep, PK), f32)
    rbft_a = np.zeros((NC, NRB, ep), bf)

    grp = core[order] * NBLK + blk[order]
    starts = np.searchsorted(grp, np.arange(NC * NBLK))
    ends = np.searchsorted(grp, np.arange(NC * NBLK) + 1)
    for c in range(NC):
        for b in range(NBLK):
            g = c * NBLK + b
            i0, i1 = int(starts[g]), int(ends[g])
            n = i1 - i0
            o = b * t_blk * 128
            senders_a[c, o:o + n] = ss[i0:i1]
            epack_a[c, o:o + n, 0] = (rs[i0:i1] - (c * NPC + b * 128)).astype(f32)
            epack_a[c, o:o + n, 1] = cut_s[i0:i1]
            epack_a[c, o:o + n, 2:18] = sh_s[i0:i1]
            rbft_a[c, :, o:o + n] = rbf_s[i0:i1].T.astype(bf)

    def wrap_pmajor(a):
        epn, cc = a.shape
        return np.ascontiguousarray(
            a.reshape(epn // 128, 128, cc).transpose(1, 0, 2).reshape(128, -1)
        )

    def bd(w):
        w = np.asarray(w, f32)
        g, d, _ = w.shape
        out = np.zeros((g * d, g * d), f32)
        for i in range(g):
            out[i * d:(i + 1) * d, i * d:(i + 1) * d] = w[i]
        return out.astype(bf)

    p = inputs
    asf = lambda k: np.asarray(p[k], f32)
    w1r = np.concatenate([asf("fi_rbf_w1"), asf("fe_rbf_w1")], 1).astype(bf)
    w1e = np.concatenate([asf("fi_ev_w1"), asf("fe_ev_w1")], 1).astype(bf)
    w2r = np.concatenate([asf("fi_rbf_w2") / ATT_I, asf("fe_rbf_w2") / ATT_E], 1).astype(bf)
    w2e_fi = np.concatenate([asf("fi_ev_w2"),
                             (asf("fi_rbf_b2") + asf("fi_ev_b2"))[None, :]], 0) / ATT_I
    w2e_fe = np.concatenate([asf("fe_ev_w2"),
                             (asf("fe_rbf_b2") + asf("fe_ev_b2"))[None, :]], 0) / ATT_E
    w2e = np.concatenate([w2e_fi, w2e_fe], 1).astype(bf)
    b1r = np.stack([asf("fi_rbf_b1"), asf("fe_rbf_b1")], 1)
    b1e = np.stack([asf("fi_ev_b1"), asf("fe_ev_b1")], 1)
    l0mk = np.zeros((EV, L), f32)
    for l in range(L):
        l0mk[SEG[l]:SEG[l + 1], l] = 1.0
    repi = np.zeros((H, 128), f32)
    for h in range(H):
        repi[h, h * DH:(h + 1) * DH] = 1.0
    repe = np.zeros((L, EV), f32)
    for l in range(L):
        repe[l, SEG[l]:SEG[l + 1]] = 1.0
    intw = asf("int_w")
    intb = asf("int_b")

    shared = {
        "xt": np.ascontiguousarray(inv.T).astype(bf),
        "evbf": wrap_pmajor(evf).astype(bf),
        "wq": bd(p["Wq_inv"]), "wk": bd(p["Wk_inv"]), "wv": bd(p["Wv_inv"]),
        "wqe": bd(p["Wq_ev"]), "wke": bd(p["Wk_ev"]),
        "w1r": w1r, "w1e": w1e, "w2r": w2r, "w2e": w2e,
        "b1r": b1r, "b1e": b1e,
        "l0m": l0mk.astype(bf), "repi": repi.astype(bf), "repe": repe.astype(bf),
        "intwh": intw[0:F, :].astype(bf),
        "intwl": np.concatenate([intw[F:F + L, :], np.zeros((28, F + L), f32), intb[None, :]], 0).astype(bf),
        "iden": np.eye(128, dtype=f32).astype(bf),
        "iotar": np.tile(np.arange(128, dtype=f32), (128, 1)).astype(bf),
        "iotac": np.arange(128, dtype=f32).reshape(128, 1).astype(bf),
    }
    in_maps = []
    for c in range(NC):
        m = dict(shared)
        m["senders"] = wrap_pmajor(senders_a[c][:, None]).astype(np.int32)
        m["epack"] = wrap_pmajor(epack_a[c].reshape(ep, PK)).astype(f32)
        m["rbft"] = np.ascontiguousarray(rbft_a[c])
        m["xtc"] = np.ascontiguousarray(inv[c * NPC:(c + 1) * NPC].T).astype(bf)
        m["invblk"] = inv[c * NPC:(c + 1) * NPC]
        m["evblk"] = evf[c * NPC:(c + 1) * NPC]
        in_maps.append(m)
    return t_blk, in_maps


_CACHE = {}


def kernel(**inputs):
    t_blk, in_maps = _host_prep(inputs)
    if t_blk not in _CACHE:
        _CACHE[t_blk] = build_program(t_blk)
    nc = _CACHE[t_blk]
    res = run_bass_kernel_spmd(nc, in_maps, core_ids=list(range(NC)))
    inv_out = np.concatenate([res.results[c]["oinv"] for c in range(NC)], 0)
    ev_out = np.concatenate([res.results[c]["oev"] for c in range(NC)], 0)
    return inv_out, ev_out


# revision 10
# speedup vs baseline: 1.0328x; 1.0328x over previous
"""EuclideanTransformer message-passing kernel for 8 trn2 cores.

Sharding: edges are partitioned by receiver block (2048 nodes/core, 16
blocks of 128 nodes per core), sorted by receiver, and padded so every
core has an identical static tile structure (T_BLK 128-edge tiles per
block). Node k/v projections are computed on-device (replicated) into a
DRAM gather table; per-edge sender rows come via indirect DMA. Receiver
q/qev values are computed per block and expanded to edges with a
one-hot matmul, so the segment-sum accumulates in PSUM per block and no
collective is needed: each core owns its receivers' output rows and the
host concatenates.
"""

import numpy as np
import ml_dtypes

import concourse.bass as bass
import concourse.mybir as mybir
from concourse.tile import TileContext
from concourse.tile_rust import add_dep_helper
from concourse.bass_utils import run_bass_kernel_spmd

BF = mybir.dt.bfloat16
F32 = mybir.dt.float32
I32 = mybir.dt.int32

N = 16384
E = 262144
F = 128
H = 8
DH = 16
L = 4
DE = 32
EV = 16
NRB = 32
SEG = [0, 1, 4, 9, 16]
NC = 8
NPC = N // NC          # nodes per core
NBLK = NPC // 128      # blocks of 128 nodes per core
ATT_I = float(np.sqrt(DH))
ATT_E = float(np.sqrt(DE))

TS_W = 400             # sender table row: k(128)|kev(128)|v(128)|ev(16)
PK = 20                # edgepack cols: recv_local, cutoff, sh(16), pad(2)


def _fix_multiwait(nc):
    """This walrus build accepts a single sync-wait per instruction; Tile
    emits several.  Hoist extras onto same-engine NoOps."""
    ctr = 0
    for f in nc.m.functions:
        for bb in f.blocks:
            new = []
            for inst in bb.instructions:
                si = getattr(inst, "sync_info", None)
                ow = list(si.on_wait) if si and si.on_wait else []
                if len(ow) > 1:
                    for w in ow[:-1]:
                        ctr += 1
                        nop = mybir.InstNoOp(
                            name=f"waitnop-{ctr}", ins=[], outs=[], engine=inst.engine
                        )
                        nop.sync_info = mybir.SyncInfo(on_wait=[w], on_update=[])
                        new.append(nop)
                    si.on_wait = [ow[-1]]
                new.append(inst)
            bb.instructions[:] = new


def build_program(t_blk):
    ep = NBLK * t_blk * 128          # padded edges per core
    AF = mybir.ActivationFunctionType

    nc = bass.Bass()
    dt = nc.dram_tensor
    senders_d = dt("senders", [128, ep // 128], I32, kind="ExternalInput")
    epack_d = dt("epack", [128, (ep // 128) * PK], F32, kind="ExternalInput")
    rbft_d = dt("rbft", [NRB, ep], BF, kind="ExternalInput")
    xt_d = dt("xt", [128, N], BF, kind="ExternalInput")
    xtc_d = dt("xtc", [128, NPC], BF, kind="ExternalInput")
    evbf_d = dt("evbf", [128, (N // 128) * EV], BF, kind="ExternalInput")
    invblk_d = dt("invblk", [NPC, F], F32, kind="ExternalInput")
    evblk_d = dt("evblk", [NPC, EV], F32, kind="ExternalInput")
    wq_d = dt("wq", [128, 128], BF, kind="ExternalInput")
    wk_d = dt("wk", [128, 128], BF, kind="ExternalInput")
    wv_d = dt("wv", [128, 128], BF, kind="ExternalInput")
    wqe_d = dt("wqe", [128, 128], BF, kind="ExternalInput")
    wke_d = dt("wke", [128, 128], BF, kind="ExternalInput")
    w1r_d = dt("w1r", [NRB, 256], BF, kind="ExternalInput")
    w1e_d = dt("w1e", [L, 64], BF, kind="ExternalInput")
    w2r_d = dt("w2r", [128, 256], BF, kind="ExternalInput")
    w2e_d = dt("w2e", [33, 256], BF, kind="ExternalInput")
    b1r_d = dt("b1r", [128, 2], F32, kind="ExternalInput")
    b1e_d = dt("b1e", [32, 2], F32, kind="ExternalInput")
    l0m_d = dt("l0m", [EV, L], BF, kind="ExternalInput")
    repi_d = dt("repi", [H, 128], BF, kind="ExternalInput")
    repe_d = dt("repe", [L, EV], BF, kind="ExternalInput")
    intwh_d = dt("intwh", [128, F + L], BF, kind="ExternalInput")
    intwl_d = dt("intwl", [33, F + L], BF, kind="ExternalInput")
    iden_d = dt("iden", [128, 128], BF, kind="ExternalInput")
    iotar_d = dt("iotar", [128, 128], BF, kind="ExternalInput")
    iotac_d = dt("iotac", [128, 1], BF, kind="ExternalInput")
    ts_d = dt("tstab", [N, TS_W], BF)
    oinv_d = dt("oinv", [NPC, F], F32, kind="ExternalOutput")
    oev_d = dt("oev", [NPC, EV], F32, kind="ExternalOutput")

    with TileContext(nc) as tc:
        with (
            tc.tile_pool(name="const", bufs=1) as cp,
            tc.tile_pool(name="sb", bufs=2) as sb,
            tc.tile_pool(name="io", bufs=2) as iop,
            tc.tile_pool(name="ps", bufs=1, space="PSUM") as ps,
        ):
            def cload(dram, shape, dtype):
                t = cp.tile(shape, dtype, tag=dram.name)
                nc.sync.dma_start(t[:], dram[:])
                return t

            wq = cload(wq_d, [128, 128], BF)
            wk = cload(wk_d, [128, 128], BF)
            wv = cload(wv_d, [128, 128], BF)
            wqe = cload(wqe_d, [128, 128], BF)
            wke = cload(wke_d, [128, 128], BF)
            w1r = cload(w1r_d, [NRB, 256], BF)
            w1e = cload(w1e_d, [L, 64], BF)
            w2r = cload(w2r_d, [128, 256], BF)
            w2e = cload(w2e_d, [33, 256], BF)
            b1r = cload(b1r_d, [128, 2], F32)
            b1e = cload(b1e_d, [32, 2], F32)
            l0m = cload(l0m_d, [EV, L], BF)
            repi = cload(repi_d, [H, 128], BF)
            repe = cload(repe_d, [L, EV], BF)
            intwh = cload(intwh_d, [128, F + L], BF)
            intwl = cload(intwl_d, [33, F + L], BF)
            iden = cload(iden_d, [128, 128], BF)
            iotar = cload(iotar_d, [128, 128], BF)
            iotac = cload(iotac_d, [128, 1], BF)
            xtc = cload(xtc_d, [128, NPC], BF)

            xt = cp.tile([128, N], BF, tag="xt_t")
            for j in range(8):
                nc.sync.dma_start(
                    xt[:, j * (N // 8):(j + 1) * (N // 8)],
                    xt_d[:, j * (N // 8):(j + 1) * (N // 8)],
                )

            # ---------- phase A: sender projection table ----------
            evb = cp.tile([128, (N // 128) * EV], BF, tag="evb_t")
            nc.sync.dma_start(evb[:], evbf_d[:])
            ts_writes = []
            ts_writes.append(nc.sync.dma_start(
                ts_d[:, 384:400].rearrange("(s p) c -> p s c", p=128),
                evb[:].rearrange("p (s c) -> p s c", c=EV),
            ))
            for g in range(N // 1024):
                tsa = sb.tile([128, 8, 384], BF, tag="tsa")
                for s in range(8):
                    nt = g * 8 + s
                    xsl = xt[:, nt * 128:(nt + 1) * 128]
                    pts = ps.tile([128, 384], F32, tag="pfw")
                    nc.tensor.matmul(out=pts[:, 0:128], lhsT=xsl, rhs=wk, start=True, stop=True)
                    nc.tensor.matmul(out=pts[:, 128:256], lhsT=xsl, rhs=wke, start=True, stop=True)
                    nc.tensor.matmul(out=pts[:, 256:384], lhsT=xsl, rhs=wv, start=True, stop=True)
                    nc.vector.tensor_copy(out=tsa[:, s, :], in_=pts[:])
                ts_writes.append(nc.sync.dma_start(
                    ts_d[g * 1024:(g + 1) * 1024, 0:384].rearrange("(s p) c -> p s c", p=128),
                    tsa[:],
                ))

            # ---------- phase B: edges ----------
            for b in range(NBLK):
                # receiver q|qev for this block, computed in place
                ptrb = ps.tile([128, 256], F32, tag="pexp")
                xcs = xtc[:, b * 128:(b + 1) * 128]
                nc.tensor.matmul(out=ptrb[:, 0:128], lhsT=xcs, rhs=wq, start=True, stop=True)
                nc.tensor.matmul(out=ptrb[:, 128:256], lhsT=xcs, rhs=wqe, start=True, stop=True)
                trch = sb.tile([128, 272], BF, tag="trch")
                nc.vector.tensor_copy(out=trch[:, 0:256], in_=ptrb[:])
                evbk = sb.tile([128, EV], F32, tag="evbk")
                nc.sync.dma_start(evbk[:], evblk_d[b * 128:(b + 1) * 128, :])
                nc.vector.tensor_copy(out=trch[:, 256:272], in_=evbk[:])

                acc = ps.tile([128, 144], F32, tag="acc")
                for tt in range(t_blk):
                    t = b * t_blk + tt
                    ch, s = t // 8, t % 8
                    if s == 0:
                        epk = iop.tile([128, 8, PK], F32, tag="epk")
                        nc.sync.dma_start(
                            epk[:],
                            epack_d[:, ch * 8 * PK:(ch + 1) * 8 * PK]
                            .rearrange("p (s c) -> p s c", c=PK),
                        )
                        snd = iop.tile([128, 8], I32, tag="snd")
                        nc.sync.dma_start(snd[:], senders_d[:, ch * 8:(ch + 1) * 8])
                        rbft = iop.tile([NRB, 1024], BF, tag="rbft")
                        nc.sync.dma_start(rbft[:], rbft_d[:, ch * 1024:(ch + 1) * 1024])

                    cut = epk[:, s, 1:2]
                    shn = epk[:, s, 2:18]
                    rbf_t = rbft[:, s * 128:(s + 1) * 128]

                    tsg = sb.tile([128, TS_W], BF, tag="tsg")
                    g_inst = nc.gpsimd.indirect_dma_start(
                        out=tsg[:], out_offset=None, in_=ts_d[:],
                        in_offset=bass.IndirectOffsetOnAxis(ap=snd[:, s:s + 1], axis=0),
                    )
                    for _w in ts_writes:
                        add_dep_helper(g_inst.ins, _w.ins, reason="gather after table write")

                    recvb = sb.tile([128, 1], BF, tag="recvb")
                    nc.vector.tensor_copy(out=recvb[:], in_=epk[:, s, 0:1])
                    ptb = ps.tile([128, 512], BF, tag="ptb")
                    nc.tensor.transpose(
                        out=ptb[:, 0:128], in_=recvb[:].to_broadcast([128, 128]),
                        identity=iden[:],
                    )
                    oh = sb.tile([128, 128], BF, tag="oh")
                    nc.vector.tensor_tensor(
                        out=oh[:], in0=recvb[:].to_broadcast([128, 128]), in1=iotar[:],
                        op=mybir.AluOpType.is_equal,
                    )
                    oht = sb.tile([128, 128], BF, tag="oht")
                    nc.vector.tensor_tensor(
                        out=oht[:], in0=iotac[:].to_broadcast([128, 128]),
                        in1=ptb[:, 0:128], op=mybir.AluOpType.is_equal,
                    )

                    pexp = ps.tile([128, 272], F32, tag="pexp")
                    nc.tensor.matmul(out=pexp[:], lhsT=oht[:], rhs=trch[:], start=True, stop=True)

                    ph1 = ps.tile([128, 384], F32, tag="ph1")
                    nc.tensor.matmul(out=ph1[:, 0:128], lhsT=w1r[:, 0:128], rhs=rbf_t, start=True, stop=True)
                    nc.tensor.matmul(out=ph1[:, 128:256], lhsT=w1r[:, 128:256], rhs=rbf_t, start=True, stop=True)
                    sfi = sb.tile([128, 128], BF, tag="sfi")
                    nc.scalar.activation(out=sfi[:], in_=ph1[:, 0:128], func=AF.Silu, bias=b1r[:, 0:1])
                    sfe = sb.tile([128, 128], BF, tag="sfe")
                    nc.scalar.activation(out=sfe[:], in_=ph1[:, 128:256], func=AF.Silu, bias=b1r[:, 1:2])

                    evd = sb.tile([128, EV], BF, tag="evd")
                    nc.vector.tensor_tensor(out=evd[:], in0=tsg[:, 384:400],
                                            in1=pexp[:, 256:272], op=mybir.AluOpType.subtract)
                    sq = sb.tile([128, EV], BF, tag="sq")
                    nc.vector.tensor_tensor(out=sq[:], in0=evd[:], in1=evd[:],
                                            op=mybir.AluOpType.mult)
                    nc.tensor.transpose(out=ptb[0:EV, 128:256], in_=sq[:], identity=iden[:])
                    sqt = sb.tile([EV, 128], BF, tag="sqt")
                    nc.vector.tensor_copy(out=sqt[:], in_=ptb[0:EV, 128:256])
                    paux = ps.tile([128, 128], F32, tag="paux")
                    nc.tensor.matmul(out=paux[0:L, :], lhsT=l0m[:], rhs=sqt[:], start=True, stop=True)
                    l0t = sb.tile([L, 128], BF, tag="l0t")
                    nc.vector.tensor_copy(out=l0t[:], in_=paux[0:L, :])
                    nc.tensor.matmul(out=ph1[0:64, 256:384], lhsT=w1e[:], rhs=l0t[:], start=True, stop=True)
                    sev = sb.tile([33, 256], BF, tag="sev")
                    nc.scalar.activation(out=sev[0:32, 0:128], in_=ph1[0:32, 256:384],
                                         func=AF.Silu, bias=b1e[:, 0:1])
                    nc.scalar.activation(out=sev[0:32, 128:256], in_=ph1[32:64, 256:384],
                                         func=AF.Silu, bias=b1e[:, 1:2])
                    nc.vector.memset(sev[32:33, 0:128], 1.0)
                    nc.vector.memset(sev[32:33, 128:256], 1.0)

                    pfw = ps.tile([128, 512], F32, tag="pfw")
                    nc.tensor.matmul(out=pfw[:, 0:128], lhsT=w2r[:, 0:128], rhs=sfi[:], start=True, stop=False)
                    nc.tensor.matmul(out=pfw[:, 0:128], lhsT=w2e[:, 0:128], rhs=sev[:, 0:128], start=False, stop=True)
                    nc.tensor.matmul(out=pfw[:, 128:256], lhsT=w2r[:, 128:256], rhs=sfe[:], start=True, stop=False)
                    nc.tensor.matmul(out=pfw[:, 128:256], lhsT=w2e[:, 128:256], rhs=sev[:, 128:256], start=False, stop=True)
                    fwi = sb.tile([128, 128], BF, tag="fwi")
                    nc.vector.tensor_copy(out=fwi[:], in_=pfw[:, 0:128])
                    fwe = sb.tile([128, 128], BF, tag="fwe")
                    nc.scalar.activation(out=fwe[:], in_=pfw[:, 128:256], func=AF.Copy)
                    pfn = ps.tile([128, 256], BF, tag="pfn")
                    nc.tensor.transpose(out=pfn[:, 0:128], in_=fwi[:], identity=iden[:])
                    nc.tensor.transpose(out=pfn[:, 128:256], in_=fwe[:], identity=iden[:])

                    t1 = sb.tile([128, 128], BF, tag="t1")
                    nc.vector.tensor_tensor(out=t1[:], in0=tsg[:, 0:128], in1=pexp[:, 0:128],
                                            op=mybir.AluOpType.mult)
                    t2 = sb.tile([128, 128], BF, tag="t2")
                    nc.vector.tensor_tensor(out=t2[:], in0=t1[:], in1=pfn[:, 0:128],
                                            op=mybir.AluOpType.mult)
                    t3 = sb.tile([128, 128], BF, tag="t3")
                    nc.vector.tensor_tensor(out=t3[:], in0=tsg[:, 128:256], in1=pexp[:, 128:256],
                                            op=mybir.AluOpType.mult)
                    t4 = sb.tile([128, 128], BF, tag="t4")
                    nc.vector.tensor_tensor(out=t4[:], in0=t3[:], in1=pfn[:, 128:256],
                                            op=mybir.AluOpType.mult)
                    acat = sb.tile([128, 12], F32, tag="acat")
                    nc.vector.reduce_sum(out=acat[:, 0:8],
                                         in_=t2[:].rearrange("p (h d) -> p h d", d=DH),
                                         axis=mybir.AxisListType.X)
                    nc.vector.reduce_sum(out=acat[:, 8:12],
                                         in_=t4[:].rearrange("p (h d) -> p h d", d=DE),
                                         axis=mybir.AxisListType.X)
                    acsi = sb.tile([128, H], BF, tag="acsi")
                    nc.vector.tensor_scalar_mul(acsi[:], acat[:, 0:8], cut)
                    acse = sb.tile([128, L], BF, tag="acse")
                    nc.vector.tensor_scalar_mul(acse[:], acat[:, 8:12], cut)
                    nc.tensor.transpose(out=ptb[0:H, 256:384], in_=acsi[:], identity=iden[:])
                    nc.tensor.transpose(out=ptb[0:L, 384:512], in_=acse[:], identity=iden[:])
                    ati = sb.tile([H, 128], BF, tag="ati")
                    nc.vector.tensor_copy(out=ati[:], in_=ptb[0:H, 256:384])
                    ate = sb.tile([L, 128], BF, tag="ate")
                    nc.vector.tensor_copy(out=ate[:], in_=ptb[0:L, 384:512])
                    prep = ps.tile([128, 144], F32, tag="prep")
                    nc.tensor.matmul(out=prep[:, 0:128], lhsT=ati[:], rhs=repi[:], start=True, stop=True)
                    nc.tensor.matmul(out=prep[:, 128:144], lhsT=ate[:], rhs=repe[:], start=True, stop=True)
                    scl = sb.tile([128, 144], BF, tag="scl")
                    nc.vector.tensor_tensor(out=scl[:, 0:128], in0=prep[:, 0:128],
                                            in1=tsg[:, 256:384], op=mybir.AluOpType.mult)
                    nc.vector.tensor_tensor(out=scl[:, 128:144], in0=prep[:, 128:144],
                                            in1=shn, op=mybir.AluOpType.mult)
                    nc.tensor.matmul(out=acc[:], lhsT=oh[:], rhs=scl[:],
                                     start=(tt == 0), stop=(tt == t_blk - 1))

                # ---- block tail: residuals + interaction ----
                invb = sb.tile([128, F], F32, tag="invb")
                nc.sync.dma_start(invb[:], invblk_d[b * 128:(b + 1) * 128, :])
                inv1 = sb.tile([128, F], F32, tag="inv1")
                nc.vector.tensor_tensor(out=inv1[:], in0=invb[:], in1=acc[:, 0:128],
                                        op=mybir.AluOpType.add)
                ev1 = sb.tile([128, EV], F32, tag="ev1")
                nc.vector.tensor_tensor(out=ev1[:], in0=evbk[:], in1=acc[:, 128:144],
                                        op=mybir.AluOpType.add)
                inv1b = sb.tile([128, F], BF, tag="inv1b")
                nc.vector.tensor_copy(out=inv1b[:], in_=inv1[:])
                pint = ps.tile([128, 512], BF, tag="ptb")
                nc.tensor.transpose(out=pint[:, 0:128], in_=inv1b[:], identity=iden[:])
                inv1t = sb.tile([128, 128], BF, tag="inv1t")
                nc.vector.tensor_copy(out=inv1t[:], in_=pint[:, 0:128])
                sq2 = sb.tile([128, EV], BF, tag="sq2")
                nc.vector.tensor_tensor(out=sq2[:], in0=ev1[:], in1=ev1[:],
                                        op=mybir.AluOpType.mult)
                nc.tensor.transpose(out=pint[0:EV, 128:256], in_=sq2[:], identity=iden[:])
                sq2t = sb.tile([EV, 128], BF, tag="sq2t")
                nc.vector.tensor_copy(out=sq2t[:], in_=pint[0:EV, 128:256])
                paux2 = ps.tile([128, 128], F32, tag="paux")
                nc.tensor.matmul(out=paux2[0:L, :], lhsT=l0m[:], rhs=sq2t[:], start=True, stop=True)
                l0b = sb.tile([33, 128], BF, tag="l0b")
                nc.vector.memset(l0b[0:32, :], 0.0)
                nc.vector.tensor_copy(out=l0b[0:L, :], in_=paux2[0:L, :])
                nc.vector.memset(l0b[32:33, :], 1.0)
                pt = ps.tile([128, F + L], F32, tag="prep")
                nc.tensor.matmul(out=pt[:], lhsT=inv1t[:], rhs=intwh[:], start=True, stop=False)
                nc.tensor.matmul(out=pt[:], lhsT=l0b[0:33, :], rhs=intwl[:], start=False, stop=True)
                oinv = sb.tile([128, F], F32, tag="oinvt")
                nc.vector.tensor_tensor(out=oinv[:], in0=inv1[:], in1=pt[:, 0:F],
                                        op=mybir.AluOpType.add)
                nc.sync.dma_start(oinv_d[b * 128:(b + 1) * 128, :], oinv[:])
                oev = sb.tile([128, EV], F32, tag="oevt")
                for l in range(L):
                    c_l = sb.tile([128, 1], F32, tag=f"cl{l}")
                    nc.scalar.add(out=c_l[:], in_=pt[:, F + l:F + l + 1], add=1.0)
                    nc.vector.tensor_scalar_mul(
                        oev[:, SEG[l]:SEG[l + 1]], ev1[:, SEG[l]:SEG[l + 1]], c_l[:],
                    )
                nc.sync.dma_start(oev_d[b * 128:(b + 1) * 128, :], oev[:])

    _fix_multiwait(nc)
    return nc


def _host_prep(inputs):
    f32 = np.float32
    bf = ml_dtypes.bfloat16
    inv = np.asarray(inputs["inv_features"], f32)
    evf = np.asarray(inputs["ev_features"], f32)
    rbf = np.asarray(inputs["rbf"], f32)
    sh = np.asarray(inputs["sh_vectors"], f32)
    cut = np.asarray(inputs["cutoffs"], f32).reshape(-1)
    snd = np.asarray(inputs["senders"]).astype(np.int64)
    rcv = np.asarray(inputs["receivers"]).astype(np.int64)

    core = rcv // NPC
    blk = (rcv % NPC) // 128
    counts = np.zeros((NC, NBLK), np.int64)
    np.add.at(counts, (core, blk), 1)
    t_blk = int(np.ceil(counts.max() / 128))
    ep = NBLK * t_blk * 128

    order = np.lexsort((rcv,))
    rs, ss = rcv[order], snd[order]
    rbf_s, sh_s, cut_s = rbf[order], sh[order], cut[order]

    senders_a = np.zeros((NC, ep), np.int32)
    epack_a = np.zeros((NC, ep, PK), f32)
    rbft_a = np.zeros((NC, NRB, ep), bf)

    grp = core[order] * NBLK + blk[order]
    starts = np.searchsorted(grp, np.arange(NC * NBLK))
    ends = np.searchsorted(grp, np.arange(NC * NBLK) + 1)
    for c in range(NC):
        for b in range(NBLK):
            g = c * NBLK + b
            i0, i1 = int(starts[g]), int(ends[g])
            n = i1 - i0
            o = b * t_blk * 128
            senders_a[c, o:o + n] = ss[i0:i1]
            epack_a[c, o:o + n, 0] = (rs[i0:i1] - (c * NPC + b * 128)).astype(f32)
            epack_a[c, o:o + n, 1] = cut_s[i0:i1]
            epack_a[c, o:o + n, 2:18] = sh_s[i0:i1]
            rbft_a[c, :, o:o + n] = rbf_s[i0:i1].T.astype(bf)

    def wrap_pmajor(a):
        epn, cc = a.shape
        return np.ascontiguousarray(
            a.reshape(epn // 128, 128, cc).transpose(1, 0, 2).reshape(128, -1)
        )

    def bd(w):
        w = np.asarray(w, f32)
        g, d, _ = w.shape
        out = np.zeros((g * d, g * d), f32)
        for i in range(g):
            out[i * d:(i + 1) * d, i * d:(i + 1) * d] = w[i]
        return out.astype(bf)

    p = inputs
    asf = lambda k: np.asarray(p[k], f32)
    w1r = np.concatenate([asf("fi_rbf_w1"), asf("fe_rbf_w1")], 1).astype(bf)
    w1e = np.concatenate([asf("fi_ev_w1"), asf("fe_ev_w1")], 1).astype(bf)
    w2r = np.concatenate([asf("fi_rbf_w2") / ATT_I, asf("fe_rbf_w2") / ATT_E], 1).astype(bf)
    w2e_fi = np.concatenate([asf("fi_ev_w2"),
                             (asf("fi_rbf_b2") + asf("fi_ev_b2"))[None, :]], 0) / ATT_I
    w2e_fe = np.concatenate([asf("fe_ev_w2"),
                             (asf("fe_rbf_b2") + asf("fe_ev_b2"))[None, :]], 0) / ATT_E
    w2e = np.concatenate([w2e_fi, w2e_fe], 1).astype(bf)
    b1r = np.stack([asf("fi_rbf_b1"), asf("fe_rbf_b1")], 1)
    b1e = np.stack([asf("fi_ev_b1"), asf("fe_ev_b1")], 1)
    l0mk = np.zeros((EV, L), f32)
    for l in range(L):
        l0mk[SEG[l]:SEG[l + 1], l] = 1.0
    repi = np.zeros((H, 128), f32)
    for h in range(H):
        repi[h, h * DH:(h + 1) * DH] = 1.0
    repe = np.zeros((L, EV), f32)
    for l in range(L):
        repe[l, SEG[l]:SEG[l + 1]] = 1.0
    intw = asf("int_w")
    intb = asf("int_b")

    shared = {
        "xt": np.ascontiguousarray(inv.T).astype(bf),
        "evbf": wrap_pmajor(evf).astype(bf),
        "wq": bd(p["Wq_inv"]), "wk": bd(p["Wk_inv"]), "wv": bd(p["Wv_inv"]),
        "wqe": bd(p["Wq_ev"]), "wke": bd(p["Wk_ev"]),
        "w1r": w1r, "w1e": w1e, "w2r": w2r, "w2e": w2e,
        "b1r": b1r, "b1e": b1e,
        "l0m": l0mk.astype(bf), "repi": repi.astype(bf), "repe": repe.astype(bf),
        "intwh": intw[0:F, :].astype(bf),
        "intwl": np.concatenate([intw[F:F + L, :], np.zeros((28, F + L), f32), intb[None, :]], 0).astype(bf),
        "iden": np.eye(128, dtype=f32).astype(bf),
        "iotar": np.tile(np.arange(128, dtype=f32), (128, 1)).astype(bf),
        "iotac": np.arange(128, dtype=f32).reshape(128, 1).astype(bf),
    }
    in_maps = []
    for c in range(NC):
        m = dict(shared)
        m["senders"] = wrap_pmajor(senders_a[c][:, None]).astype(np.int32)
        m["epack"] = wrap_pmajor(epack_a[c].reshape(ep, PK)).astype(f32)
        m["rbft"] = np.ascontiguousarray(rbft_a[c])
        m["xtc"] = np.ascontiguousarray(inv[c * NPC:(c + 1) * NPC].T).astype(bf)
        m["invblk"] = inv[c * NPC:(c + 1) * NPC]
        m["evblk"] = evf[c * NPC:(c + 1) * NPC]
        in_maps.append(m)
    return t_blk, in_maps


_CACHE = {}


def kernel(**inputs):
    t_blk, in_maps = _host_prep(inputs)
    if t_blk not in _CACHE:
        _CACHE[t_blk] = build_program(t_blk)
    nc = _CACHE[t_blk]
    res = run_bass_kernel_spmd(nc, in_maps, core_ids=list(range(NC)))
    inv_out = np.concatenate([res.results[c]["oinv"] for c in range(NC)], 0)
    ev_out = np.concatenate([res.results[c]["oev"] for c in range(NC)], 0)
    return inv_out, ev_out
